# revision 10
# baseline (speedup 1.0000x reference)
"""Expert-parallel MoE MLP (BaseMLPExperts) for 8 TRN2 NeuronCores.

Reference computation (per expert e):
    y[:, e, :] = gelu_exact(x[:, e, :] @ wi[e]) @ wo[e]
with T=8192 tokens, E=8 experts, H=1024 hidden, I=4096 intermediate, fp32.

Sharding: expert-parallel — core e owns expert e (its x slice, wi[e], wo[e]).
No cross-core communication.

Per-core device kernel (all matmuls in f32r = TF32-on-PE at full PE rate,
fp32 PSUM accumulation; measured rel-err ~1.5e-4 for K=1024):
  Phase 1: h1T[I, T] = gelu(wi.T-free GEMM over xT), streamed by 512-token
           tiles; wi ([128p, 8, 4096] = 128KB/partition) SBUF-resident;
           GELU (exact erf form) applied on PSUM eviction by the ACT engine,
           written to DRAM scratch as f32r.
  Phase 2: y[T, H] = h1 @ wo, streamed by 128-token blocks; wo
           ([128p, 32, 1024] = 128KB/partition) SBUF-resident; h1T tiles act
           as the stationary matmul operand so y comes out untransposed.

Host side: transposes x slices to xT (H-major), shards, runs the SPMD kernel
on cores 0-7, stacks per-core y into [T, E, H].
"""

import numpy as np

import concourse.bass as bass
import concourse.mybir as mybir
import concourse.tile as tile
from concourse import bacc
from concourse.bass_utils import run_bass_kernel_spmd

T, E, H, I = 8192, 8, 1024, 4096
P = 128
F32 = mybir.dt.float32
F32R = mybir.dt.float32r

TT1 = 512            # phase-1 token tile
NT1 = T // TT1       # 16
HT = H // P          # 8 k-tiles for GEMM1
IT = I // P          # 32 i-tiles
TT2 = 128            # phase-2 token block
NT2 = T // TT2       # 64

# run_bass_kernel_spmd kwargs injected by test harness (e.g. trace=True)
RUN_KWARGS: dict = {}
LAST_RESULT = None

_NC = None


def _build():
    nc = bacc.Bacc("TRN2", target_bir_lowering=False, debug=False, num_devices=8)

    xT = nc.dram_tensor("xT", [H, T], F32R, kind="ExternalInput").ap()
    wi = nc.dram_tensor("wi", [H, I], F32R, kind="ExternalInput").ap()
    wo = nc.dram_tensor("wo", [I, H], F32R, kind="ExternalInput").ap()
    y = nc.dram_tensor("y", [T, H], F32, kind="ExternalOutput").ap()

    xT_r = xT.rearrange("(ho p) t -> p ho t", p=P)      # [128, 8, T]
    wi_r = wi.rearrange("(ho p) i -> p ho i", p=P)      # [128, 8, I]
    wo_r = wo.rearrange("(io p) h -> p io h", p=P)      # [128, 32, H]

    with tile.TileContext(nc) as tc:
        with tc.tile_pool(name="h1dram", bufs=1, space="DRAM") as dpool:
            # h1T scratch: one [I, TT1] block per phase-1 token tile
            h1b = [
                dpool.tile([I, TT1], F32R, name=f"h1b{t}", tag=f"h1b{t}")
                for t in range(NT1)
            ]

            # wo i-tiles 0..7 prefetched during phase 1 (32KB/p headroom)
            wo_pre_pool = tc.alloc_tile_pool(name="wo_pre_pool", bufs=1)
            wo_pre = wo_pre_pool.tile([P, 8, H], F32R, name="wo_pre")
            for g in range(2):
                nc.gpsimd.dma_start(
                    out=wo_pre[:, 4 * g : 4 * g + 4, :],
                    in_=wo_r[:, 4 * g : 4 * g + 4, :],
                )

            # ---------------- Phase 1: h1T = gelu(x @ wi), transposed ----
            with (
                tc.tile_pool(name="wi_pool", bufs=1) as wi_pool,
                tc.tile_pool(name="xt_pool", bufs=2) as xt_pool,
                tc.tile_pool(name="h1o_pool", bufs=6) as h1o_pool,
                tc.tile_pool(name="ps1_pool", bufs=8, space="PSUM") as ps1_pool,
            ):
                # wi loaded in 8 i-chunks of 2MB (all h per chunk), ordered
                # like phase-1 consumption so the first matmul group only
                # waits for chunk 0; issue spread over two otherwise-idle
                # engine queues
                wi_sb = wi_pool.tile([P, HT, I], F32R, name="wi_sb")
                for g in range(8):
                    eng = nc.sync if g % 2 == 0 else nc.gpsimd
                    eng.dma_start(
                        out=wi_sb[:, :, g * 512 : (g + 1) * 512],
                        in_=wi_r[:, :, g * 512 : (g + 1) * 512],
                    )

                for tt in range(NT1):
                    t0 = tt * TT1
                    xt = xt_pool.tile([P, HT, TT1], F32R, name="xt", tag="xt")
                    for g in range(2):
                        nc.sync.dma_start(
                            out=xt[:, 4 * g : 4 * g + 4, :],
                            in_=xT_r[:, 4 * g : 4 * g + 4, t0 : t0 + TT1],
                        )
                    for i in range(IT):
                        ps = ps1_pool.tile([P, TT1], F32, name="ps1", tag="ps1")
                        for h in range(HT):
                            nc.tensor.matmul(
                                ps[:],
                                wi_sb[:, h, i * P : (i + 1) * P],
                                xt[:, h, :],
                                start=(h == 0),
                                stop=(h == HT - 1),
                            )
                        h1o = h1o_pool.tile([P, TT1], F32R, name="h1o", tag="h1o")
                        nc.scalar.activation(
                            h1o[:], ps[:], mybir.ActivationFunctionType.Gelu
                        )
                        nc.gpsimd.dma_start(
                            out=h1b[tt][i * P : (i + 1) * P, :], in_=h1o[:]
                        )

            # ---------------- Phase 2: y = h1 @ wo ----------------------
            with (
                tc.tile_pool(name="wo_pool", bufs=1) as wo_pool,
                tc.tile_pool(name="h1i_pool", bufs=3) as h1i_pool,
                tc.tile_pool(name="yo_pool", bufs=4) as yo_pool,
                tc.tile_pool(name="ps2_pool", bufs=8, space="PSUM") as ps2_pool,
            ):
                # i-tiles 8..31 land here as wi's SBUF frees; loaded in
                # 4-tile chunks in consumption order across three queues
                wo_sb = wo_pool.tile([P, IT - 8, H], F32R, name="wo_sb")
                wo_engs = [nc.sync, nc.gpsimd, nc.scalar]
                for g in range(6):
                    wo_engs[g % 3].dma_start(
                        out=wo_sb[:, 4 * g : 4 * g + 4, :],
                        in_=wo_r[:, 8 + 4 * g : 8 + 4 * g + 4, :],
                    )

                def wo_slice(i, hh):
                    if i < 8:
                        return wo_pre[:, i, hh * 512 : (hh + 1) * 512]
                    return wo_sb[:, i - 8, hh * 512 : (hh + 1) * 512]

                for tb in range(NT2):
                    tt, tsub = tb // 4, tb % 4
                    src = h1b[tt].rearrange("(io p) t -> p io t", p=P)
                    h1i = h1i_pool.tile([P, IT, TT2], F32R, name="h1i", tag="h1i")
                    for g in range(4):
                        eng = nc.sync if g % 2 == 0 else nc.gpsimd
                        eng.dma_start(
                            out=h1i[:, 8 * g : 8 * g + 8, :],
                            in_=src[
                                :, 8 * g : 8 * g + 8, tsub * TT2 : (tsub + 1) * TT2
                            ],
                        )
                    yo = yo_pool.tile([P, H], F32, name="yo", tag="yo")
                    for hh in range(2):
                        ps = ps2_pool.tile([P, 512], F32, name="ps2", tag="ps2")
                        for i in range(IT):
                            nc.tensor.matmul(
                                ps[:],
                                h1i[:, i, :],
                                wo_slice(i, hh),
                                start=(i == 0),
                                stop=(i == IT - 1),
                            )
                        nc.vector.tensor_copy(yo[:, hh * 512 : (hh + 1) * 512], ps[:])
                    nc.scalar.dma_start(
                        out=y[tb * TT2 : (tb + 1) * TT2, :], in_=yo[:]
                    )
            wo_pre_pool.release()

    nc.compile()
    return nc


def kernel(x: np.ndarray, wi: np.ndarray, wo: np.ndarray) -> np.ndarray:
    global _NC, LAST_RESULT
    x = np.asarray(x, dtype=np.float32)
    wi = np.asarray(wi, dtype=np.float32)
    wo = np.asarray(wo, dtype=np.float32)
    assert x.shape == (T, E, H) and wi.shape == (E, H, I) and wo.shape == (E, I, H)

    if _NC is None:
        _NC = _build()

    in_maps = [
        {
            "xT": np.ascontiguousarray(x[:, e, :].T),
            "wi": np.ascontiguousarray(wi[e]),
            "wo": np.ascontiguousarray(wo[e]),
        }
        for e in range(E)
    ]
    res = run_bass_kernel_spmd(
        _NC, in_maps, core_ids=list(range(E)), **RUN_KWARGS
    )
    LAST_RESULT = res
    out = np.stack([res.results[e]["y"] for e in range(E)], axis=1)
    return np.ascontiguousarray(out.astype(np.float32, copy=False))


# revision 15
# speedup vs baseline: 1.0222x; 1.0222x over previous
"""Expert-parallel MoE MLP (BaseMLPExperts) for 8 TRN2 NeuronCores.

Reference computation (per expert e):
    y[:, e, :] = gelu_exact(x[:, e, :] @ wi[e]) @ wo[e]
with T=8192 tokens, E=8 experts, H=1024 hidden, I=4096 intermediate, fp32.

Sharding: expert-parallel — core e owns expert e (its x slice, wi[e], wo[e]).
No cross-core communication.

Per-core device kernel (all matmuls in f32r = TF32-on-PE at full PE rate,
fp32 PSUM accumulation; measured rel-err ~2e-4 end to end):
  Phase 1: h1T[I, T] = gelu(x @ wi) transposed, streamed by 512-token tiles;
           wi SBUF-resident (128KB/partition, split into lo/hi halves so the
           last token tile can release them in stages); GELU (exact erf form)
           applied on PSUM eviction by the ACT engine, written to DRAM
           scratch as f32r.
  Phase 2: y[T, H] = h1 @ wo, streamed by 128-token blocks; wo SBUF-resident
           (prefetched in three pieces: 8 i-tiles during phase 1, 16 as
           wi_hi's space frees, 8 as wi_lo's space frees); h1T tiles act as
           the stationary matmul operand so y comes out untransposed.

DMA issue queues (SP/GpSimd/ACT sequencers) are spread and emission-ordered
so the first matmul group only waits for ~4MB of priming traffic.

Host side: transposes x slices to xT (H-major), shards, runs the SPMD kernel
on cores 0-7, stacks per-core y into [T, E, H].
"""

import numpy as np

import concourse.bass as bass
import concourse.mybir as mybir
import concourse.tile as tile
from concourse import bacc
from concourse.bass_utils import run_bass_kernel_spmd

T, E, H, I = 8192, 8, 1024, 4096
P = 128
F32 = mybir.dt.float32
F32R = mybir.dt.float32r

TT1 = 512            # phase-1 token tile
NT1 = T // TT1       # 16
HT = H // P          # 8 k-tiles for GEMM1
IT = I // P          # 32 i-tiles
TT2 = 128            # phase-2 token block
NT2 = T // TT2       # 64

# run_bass_kernel_spmd kwargs injected by test harness (e.g. trace=True)
RUN_KWARGS: dict = {}
LAST_RESULT = None

_NC = None


def _build():
    nc = bacc.Bacc("TRN2", target_bir_lowering=False, debug=False, num_devices=8)

    xT = nc.dram_tensor("xT", [H, T], F32R, kind="ExternalInput").ap()
    wi = nc.dram_tensor("wi", [H, I], F32R, kind="ExternalInput").ap()
    wo = nc.dram_tensor("wo", [I, H], F32R, kind="ExternalInput").ap()
    y = nc.dram_tensor("y", [T, H], F32, kind="ExternalOutput").ap()

    xT_r = xT.rearrange("(ho p) t -> p ho t", p=P)      # [128, 8, T]
    wi_r = wi.rearrange("(ho p) i -> p ho i", p=P)      # [128, 8, I]
    wo_r = wo.rearrange("(io p) h -> p io h", p=P)      # [128, 32, H]

    with tile.TileContext(nc) as tc:
        with tc.tile_pool(name="h1dram", bufs=1, space="DRAM") as dpool:
            # h1T scratch: one [I, TT1] block per phase-1 token tile
            h1b = [
                dpool.tile([I, TT1], F32R, name=f"h1b{t}", tag=f"h1b{t}")
                for t in range(NT1)
            ]

            # wi lives in two 64KB/partition tiles; after their last phase-1
            # read, the SAME storage is refilled with wo via rearranged views
            # (Tile's subtile tracker serializes the WAR), so the bulk of wo
            # streams in under phase-1's tail instead of stalling phase 2.
            wo_pre_pool = tc.alloc_tile_pool(name="wo_pre_pool", bufs=1)
            wi_pool = tc.alloc_tile_pool(name="wi_pool", bufs=1)
            wo_pre = wo_pre_pool.tile([P, 8, H], F32R, name="wo_pre")
            wi_lo = wi_pool.tile([P, HT, I // 2], F32R, name="wi_lo")
            wi_hi = wi_pool.tile([P, HT, I // 2], F32R, name="wi_hi")
            # wo views aliasing wi storage: wo_mid = i-tiles 8..23 (in wi_hi),
            # wo_end = i-tiles 24..31 (in first half of wi_lo)
            wo_mid = wi_hi.rearrange("p h c -> p (h c)").rearrange(
                "p (a b) -> p a b", b=H
            )
            wo_end = wi_lo[:, : HT // 2, :].rearrange("p h c -> p (h c)").rearrange(
                "p (a b) -> p a b", b=H
            )

            def wi_slice(h, i):
                if i < 16:
                    return wi_lo[:, h, i * P : (i + 1) * P]
                return wi_hi[:, h, (i - 16) * P : (i - 15) * P]

            with (
                tc.tile_pool(name="xt_pool", bufs=2) as xt_pool,
                tc.tile_pool(name="h1o_pool", bufs=6) as h1o_pool,
                tc.tile_pool(name="ps1_pool", bufs=8, space="PSUM") as ps1_pool,
            ):
                def load_xt(tt):
                    t0 = tt * TT1
                    xt = xt_pool.tile([P, HT, TT1], F32R, name="xt", tag="xt")
                    for g in range(2):
                        nc.sync.dma_start(
                            out=xt[:, 4 * g : 4 * g + 4, :],
                            in_=xT_r[:, 4 * g : 4 * g + 4, t0 : t0 + TT1],
                        )
                    return xt

                # Priming order: xt(tt=0) on SP, wi chunk 0 on GpSimd, then
                # the remaining wi chunks in consumption order, wo_pre last.
                xt_cur = load_xt(0)
                for g in range(8):
                    dst = wi_lo if g < 4 else wi_hi
                    c0 = (g % 4) * 512
                    eng = nc.gpsimd if g % 2 == 0 else nc.sync
                    eng.dma_start(
                        out=dst[:, :, c0 : c0 + 512],
                        in_=wi_r[:, :, g * 512 : (g + 1) * 512],
                    )
                for g in range(2):
                    nc.gpsimd.dma_start(
                        out=wo_pre[:, 4 * g : 4 * g + 4, :],
                        in_=wo_r[:, 4 * g : 4 * g + 4, :],
                    )

                def igroup(tt, i, xt):
                    ps = ps1_pool.tile([P, TT1], F32, name="ps1", tag="ps1")
                    for h in range(HT):
                        nc.tensor.matmul(
                            ps[:],
                            wi_slice(h, i),
                            xt[:, h, :],
                            start=(h == 0),
                            stop=(h == HT - 1),
                        )
                    h1o = h1o_pool.tile([P, TT1], F32R, name="h1o", tag="h1o")
                    nc.scalar.activation(
                        h1o[:], ps[:], mybir.ActivationFunctionType.Gelu
                    )
                    nc.gpsimd.dma_start(
                        out=h1b[tt][i * P : (i + 1) * P, :], in_=h1o[:]
                    )

                for tt in range(NT1 - 1):
                    xt_nxt = load_xt(tt + 1)
                    for i in range(IT):
                        igroup(tt, i, xt_cur)
                    xt_cur = xt_nxt

                # Last token tile: consume wi_hi's i-groups first so its
                # storage frees early, letting the bulk of wo stream in under
                # the phase-1 tail (writes alias wi_hi; WAR auto-serialized).
                for i in range(16, IT):
                    igroup(NT1 - 1, i, xt_cur)

                for g in range(4):
                    eng = nc.sync if g % 2 == 0 else nc.gpsimd
                    eng.dma_start(
                        out=wo_mid[:, 4 * g : 4 * g + 4, :],
                        in_=wo_r[:, 8 + 4 * g : 8 + 4 * g + 4, :],
                    )

                for i in range(16):
                    igroup(NT1 - 1, i, xt_cur)

            # ---------------- Phase 2: y = h1 @ wo ----------------------
            for g in range(2):
                eng = nc.sync if g % 2 == 0 else nc.gpsimd
                eng.dma_start(
                    out=wo_end[:, 4 * g : 4 * g + 4, :],
                    in_=wo_r[:, 24 + 4 * g : 24 + 4 * g + 4, :],
                )

            def wo_slice(i, hh):
                hs = slice(hh * 512, (hh + 1) * 512)
                if i < 8:
                    return wo_pre[:, i, hs]
                if i < 24:
                    return wo_mid[:, i - 8, hs]
                return wo_end[:, i - 24, hs]

            with (
                tc.tile_pool(name="h1i_pool", bufs=2) as h1i_pool,
                tc.tile_pool(name="yo_pool", bufs=3) as yo_pool,
                tc.tile_pool(name="ps2_pool", bufs=8, space="PSUM") as ps2_pool,
            ):
                for tb in range(NT2):
                    tt, tsub = tb // 4, tb % 4
                    src = h1b[tt].rearrange("(io p) t -> p io t", p=P)
                    h1i = h1i_pool.tile([P, IT, TT2], F32R, name="h1i", tag="h1i")
                    for g in range(4):
                        eng = nc.sync if g % 2 == 0 else nc.gpsimd
                        eng.dma_start(
                            out=h1i[:, 8 * g : 8 * g + 8, :],
                            in_=src[
                                :, 8 * g : 8 * g + 8, tsub * TT2 : (tsub + 1) * TT2
                            ],
                        )
                    yo = yo_pool.tile([P, H], F32, name="yo", tag="yo")
                    for hh in range(2):
                        ps = ps2_pool.tile([P, 512], F32, name="ps2", tag="ps2")
                        for i in range(IT):
                            nc.tensor.matmul(
                                ps[:],
                                h1i[:, i, :],
                                wo_slice(i, hh),
                                start=(i == 0),
                                stop=(i == IT - 1),
                            )
                        nc.vector.tensor_copy(yo[:, hh * 512 : (hh + 1) * 512], ps[:])
                    nc.scalar.dma_start(
                        out=y[tb * TT2 : (tb + 1) * TT2, :], in_=yo[:]
                    )
            wi_pool.release()
            wo_pre_pool.release()

    nc.compile()
    return nc


def kernel(x: np.ndarray, wi: np.ndarray, wo: np.ndarray) -> np.ndarray:
    global _NC, LAST_RESULT
    x = np.asarray(x, dtype=np.float32)
    wi = np.asarray(wi, dtype=np.float32)
    wo = np.asarray(wo, dtype=np.float32)
    assert x.shape == (T, E, H) and wi.shape == (E, H, I) and wo.shape == (E, I, H)

    if _NC is None:
        _NC = _build()

    in_maps = [
        {
            "xT": np.ascontiguousarray(x[:, e, :].T),
            "wi": np.ascontiguousarray(wi[e]),
            "wo": np.ascontiguousarray(wo[e]),
        }
        for e in range(E)
    ]
    res = run_bass_kernel_spmd(
        _NC, in_maps, core_ids=list(range(E)), **RUN_KWARGS
    )
    LAST_RESULT = res
    out = np.stack([res.results[e]["y"] for e in range(E)], axis=1)
    return np.ascontiguousarray(out.astype(np.float32, copy=False))


# revision 20
# speedup vs baseline: 1.0297x; 1.0073x over previous
"""Expert-parallel MoE MLP (BaseMLPExperts) for 8 TRN2 NeuronCores.

Reference computation (per expert e):
    y[:, e, :] = gelu_exact(x[:, e, :] @ wi[e]) @ wo[e]
with T=8192 tokens, E=8 experts, H=1024 hidden, I=4096 intermediate, fp32.

Sharding: expert-parallel — core e owns expert e (its x slice, wi[e], wo[e]).
No cross-core communication.

Per-core device kernel (all matmuls in f32r = TF32-on-PE at full PE rate,
fp32 PSUM accumulation; measured rel-err ~2e-4 end to end):
  Phase 1: h1T[I, T] = gelu(x @ wi) transposed, streamed by 512-token tiles;
           wi SBUF-resident (128KB/partition, split into lo/hi halves so the
           last token tile can release them in stages); GELU (exact erf form)
           applied on PSUM eviction by the ACT engine, written to DRAM
           scratch as f32r.
  Phase 2: y[T, H] = h1 @ wo, streamed by 128-token blocks; wo SBUF-resident
           (prefetched in three pieces: 8 i-tiles during phase 1, 16 as
           wi_hi's space frees, 8 as wi_lo's space frees); h1T tiles act as
           the stationary matmul operand so y comes out untransposed.

DMA issue queues (SP/GpSimd/ACT sequencers) are spread and emission-ordered
so the first matmul group only waits for ~4MB of priming traffic.

Host side: transposes x slices to xT (H-major), shards, runs the SPMD kernel
on cores 0-7, stacks per-core y into [T, E, H].
"""

import numpy as np

import concourse.bass as bass
import concourse.mybir as mybir
import concourse.tile as tile
from concourse import bacc
from concourse.bass_utils import run_bass_kernel_spmd

T, E, H, I = 8192, 8, 1024, 4096
P = 128
F32 = mybir.dt.float32
F32R = mybir.dt.float32r

TT1 = 512            # phase-1 token tile
NT1 = T // TT1       # 16
HT = H // P          # 8 k-tiles for GEMM1
IT = I // P          # 32 i-tiles
TT2 = 128            # phase-2 token block
NT2 = T // TT2       # 64

# run_bass_kernel_spmd kwargs injected by test harness (e.g. trace=True)
RUN_KWARGS: dict = {}
LAST_RESULT = None

_NC = None


def _build():
    nc = bacc.Bacc("TRN2", target_bir_lowering=False, debug=False, num_devices=8)

    xT = nc.dram_tensor("xT", [H, T], F32R, kind="ExternalInput").ap()
    wi = nc.dram_tensor("wi", [H, I], F32R, kind="ExternalInput").ap()
    wo = nc.dram_tensor("wo", [I, H], F32R, kind="ExternalInput").ap()
    y = nc.dram_tensor("y", [T, H], F32, kind="ExternalOutput").ap()

    xT_r = xT.rearrange("(ho p) t -> p ho t", p=P)      # [128, 8, T]
    wi_r = wi.rearrange("(ho p) i -> p ho i", p=P)      # [128, 8, I]
    wo_r = wo.rearrange("(io p) h -> p io h", p=P)      # [128, 32, H]

    with tile.TileContext(nc) as tc:
        with tc.tile_pool(name="h1dram", bufs=1, space="DRAM") as dpool:
            # h1T scratch: one [I, TT1] block per phase-1 token tile
            h1b = [
                dpool.tile([I, TT1], F32R, name=f"h1b{t}", tag=f"h1b{t}")
                for t in range(NT1)
            ]

            # wi lives in two 64KB/partition tiles; the last token tile
            # consumes wi chunk by chunk (512 i-columns each), and each
            # freed 16KB chunk space is immediately refilled with a 4-i-tile
            # piece of wo via matching 4D APs (Tile's subtile tracker
            # serializes the WAR), so wo streams in under phase-1's tail.
            wo_pre_pool = tc.alloc_tile_pool(name="wo_pre_pool", bufs=1)
            wi_pool = tc.alloc_tile_pool(name="wi_pool", bufs=1)
            wo_pre = wo_pre_pool.tile([P, 8, H], F32R, name="wo_pre")
            wi_lo = wi_pool.tile([P, HT, I // 2], F32R, name="wi_lo")
            wi_hi = wi_pool.tile([P, HT, I // 2], F32R, name="wi_hi")

            def wi_slice(h, i):
                if i < 16:
                    return wi_lo[:, h, i * P : (i + 1) * P]
                return wi_hi[:, h, (i - 16) * P : (i - 15) * P]

            def wi_chunk_space(q):
                # 16KB/partition column range of wi chunk q (i-cols q*512..)
                t = wi_lo if q < 4 else wi_hi
                return t[:, :, (q % 4) * 512 : (q % 4 + 1) * 512]

            # tt15 processes wi chunks in this order; wo piece k (i-tiles
            # 8+4k .. 11+4k) lands in the space of chunk WO_DEST[k]
            TT15_ORDER = [4, 5, 6, 7, 0, 1, 2, 3]
            WO_DEST = [4, 5, 6, 7, 0, 1]

            def load_wo_piece(k, eng0, eng1):
                i0 = 8 + 4 * k
                dst4 = wi_chunk_space(WO_DEST[k]).rearrange(
                    "p (a s) c -> p s a c", s=2
                )
                for s, eng in ((0, eng0), (1, eng1)):
                    eng.dma_start(
                        out=dst4[:, s],
                        in_=wo_r[:, i0 : i0 + 4, s * 512 : (s + 1) * 512],
                    )

            def wo_slice(i, hh):
                if i < 8:
                    return wo_pre[:, i, hh * 512 : (hh + 1) * 512]
                k, j = (i - 8) // 4, (i - 8) % 4
                return wi_chunk_space(WO_DEST[k])[:, 2 * j + hh, :]

            with (
                tc.tile_pool(name="xt_pool", bufs=2) as xt_pool,
                tc.tile_pool(name="h1o_pool", bufs=6) as h1o_pool,
                tc.tile_pool(name="ps1_pool", bufs=8, space="PSUM") as ps1_pool,
            ):
                def load_xt(tt):
                    t0 = tt * TT1
                    xt = xt_pool.tile([P, HT, TT1], F32R, name="xt", tag="xt")
                    for g in range(2):
                        nc.sync.dma_start(
                            out=xt[:, 4 * g : 4 * g + 4, :],
                            in_=xT_r[:, 4 * g : 4 * g + 4, t0 : t0 + TT1],
                        )
                    return xt

                # Priming order: xt(tt=0) on SP, wi chunk 0 on GpSimd, then
                # the remaining wi chunks in consumption order. wo_pre is
                # emitted later (after tt0) so it doesn't delay wi.
                xt_cur = load_xt(0)
                for g in range(8):
                    eng = nc.gpsimd if g % 2 == 0 else nc.sync
                    eng.dma_start(
                        out=wi_chunk_space(g),
                        in_=wi_r[:, :, g * 512 : (g + 1) * 512],
                    )

                def igroup(tt, i, xt):
                    ps = ps1_pool.tile([P, TT1], F32, name="ps1", tag="ps1")
                    for h in range(HT):
                        nc.tensor.matmul(
                            ps[:],
                            wi_slice(h, i),
                            xt[:, h, :],
                            start=(h == 0),
                            stop=(h == HT - 1),
                        )
                    h1o = h1o_pool.tile([P, TT1], F32R, name="h1o", tag="h1o")
                    nc.scalar.activation(
                        h1o[:], ps[:], mybir.ActivationFunctionType.Gelu
                    )
                    nc.gpsimd.dma_start(
                        out=h1b[tt][i * P : (i + 1) * P, :], in_=h1o[:]
                    )

                for tt in range(NT1 - 1):
                    xt_nxt = load_xt(tt + 1)
                    for i in range(IT):
                        igroup(tt, i, xt_cur)
                    if tt == 0:
                        for g in range(2):
                            nc.gpsimd.dma_start(
                                out=wo_pre[:, 4 * g : 4 * g + 4, :],
                                in_=wo_r[:, 4 * g : 4 * g + 4, :],
                            )
                    xt_cur = xt_nxt

                # Last token tile: consume wi chunk by chunk; right after a
                # chunk's last read, stream the matching wo piece into its
                # space (WAR auto-serialized by the subtile tracker).
                for n, q in enumerate(TT15_ORDER):
                    for i in range(4 * q, 4 * q + 4):
                        igroup(NT1 - 1, i, xt_cur)
                    if n < len(WO_DEST):
                        load_wo_piece(n, nc.sync, nc.gpsimd)

            # ---------------- Phase 2: y = h1 @ wo ----------------------
            with (
                tc.tile_pool(name="h1i_pool", bufs=2) as h1i_pool,
                tc.tile_pool(name="yo_pool", bufs=3) as yo_pool,
                tc.tile_pool(name="ps2_pool", bufs=8, space="PSUM") as ps2_pool,
            ):
                for tb in range(NT2):
                    tt, tsub = tb // 4, tb % 4
                    src = h1b[tt].rearrange("(io p) t -> p io t", p=P)
                    h1i = h1i_pool.tile([P, IT, TT2], F32R, name="h1i", tag="h1i")
                    for g in range(4):
                        eng = nc.sync if g % 2 == 0 else nc.gpsimd
                        eng.dma_start(
                            out=h1i[:, 8 * g : 8 * g + 8, :],
                            in_=src[
                                :, 8 * g : 8 * g + 8, tsub * TT2 : (tsub + 1) * TT2
                            ],
                        )
                    yo = yo_pool.tile([P, H], F32, name="yo", tag="yo")
                    for hh in range(2):
                        ps = ps2_pool.tile([P, 512], F32, name="ps2", tag="ps2")
                        for i in range(IT):
                            nc.tensor.matmul(
                                ps[:],
                                h1i[:, i, :],
                                wo_slice(i, hh),
                                start=(i == 0),
                                stop=(i == IT - 1),
                            )
                        nc.vector.tensor_copy(yo[:, hh * 512 : (hh + 1) * 512], ps[:])
                    nc.scalar.dma_start(
                        out=y[tb * TT2 : (tb + 1) * TT2, :], in_=yo[:]
                    )
            wi_pool.release()
            wo_pre_pool.release()

    nc.compile()
    return nc


def kernel(x: np.ndarray, wi: np.ndarray, wo: np.ndarray) -> np.ndarray:
    global _NC, LAST_RESULT
    x = np.asarray(x, dtype=np.float32)
    wi = np.asarray(wi, dtype=np.float32)
    wo = np.asarray(wo, dtype=np.float32)
    assert x.shape == (T, E, H) and wi.shape == (E, H, I) and wo.shape == (E, I, H)

    if _NC is None:
        _NC = _build()

    in_maps = [
        {
            "xT": np.ascontiguousarray(x[:, e, :].T),
            "wi": np.ascontiguousarray(wi[e]),
            "wo": np.ascontiguousarray(wo[e]),
        }
        for e in range(E)
    ]
    res = run_bass_kernel_spmd(
        _NC, in_maps, core_ids=list(range(E)), **RUN_KWARGS
    )
    LAST_RESULT = res
    out = np.stack([res.results[e]["y"] for e in range(E)], axis=1)
    return np.ascontiguousarray(out.astype(np.float32, copy=False))
